# revision 25
# baseline (speedup 1.0000x reference)
"""GCN block (3 layers) on 8 trn2 NeuronCores, data-parallel over batch.

Math: each layer is X' = (adj + I) @ lrelu(X @ W).
Fold each layer's weight into the previous layer's output (A(HW) = (AH)W)
so every layer is one adjacency matmul plus an identity add:

    H0 = lrelu(X0 W0)
    layer l:  G_l = H_l W_{l+1}   (W3 := I)
              Z   = adj @ G_l + G_l
              H_{l+1} = lrelu(Z)   (no lrelu after layer 2)

Precision/bandwidth: adj entries are uniform in [0, 2/N], tiny relative
to the identity term, so the adjacency product tolerates fp8. We store
at8 = fp8_e4m3(S * adj^T) with S=2048 (entries in [0,1]) — 16 MB — fully
SBUF-resident (streamed from HBM exactly once), and fp8 DoubleRow runs
the PE at 2x bf16 (256-deep contraction per instruction). The identity
term keeps fp16 precision: one extra fp16 matmul into the same PSUM
bank, so PSUM holds S*(adj@G + G). H is carried as S*H in fp16 (the
descale folds into the weight slots).

Dataflow: A^T is streamed as 8 COLUMN panels (pre-tiled on the host so
each panel is one fully-contiguous 16KB-per-partition DMA). A column
panel j carries ALL of the contraction for output chunk j, so layer 0's
output chunks complete incrementally as panels arrive — and a layer-1
WAVEFRONT accumulates (pair p, chunk c) contributions into 5 persistent
PSUM banks during the stream, filling the PE idle time of the DMA-bound
phase. Remaining layer-1 chunks and layer 2 run chunk-major from the
resident panels afterwards.

Per core: 8 samples x 16 features = 128 = partition width. Layouts:
    T-layout  [c=(b,d), m]   (128 partitions, N free)  for H (= S*H f16)
    N-layout  [m(part), mt, c]                          for G (fp8)
    panel j   [k%128, k//128 (32 k-tiles), 512 cols]    for A^T (fp8)
"""

import numpy as np

N_FULL = 4096
D = 16
B_FULL = 64
NCORES = 8
B_CORE = B_FULL // NCORES  # 8
C = B_CORE * D  # 128 partitions
P = 128
NEG_SLOPE = 0.2
SCALE = 2048.0

_CACHE = {}


def _build_nc(n, free, use_double_row=True, wavefront=5):
    import concourse.mybir as mybir
    import concourse.tile as tile
    from concourse import bacc

    f32 = mybir.dt.float32
    f16 = mybir.dt.float16
    f8 = mybir.dt.float8e4
    u8 = mybir.dt.uint8
    ALU = mybir.AluOpType
    DR = mybir.MatmulPerfMode.DoubleRow if use_double_row else None

    nt = n // P          # 32 m-tiles
    nch = n // free      # 8 column chunks == column panels
    npr = nt // 2        # 16 DoubleRow k-pairs
    tpc = nt // nch      # 4 m-tiles per chunk
    nwb = wavefront      # persistent L1 wavefront banks

    nc = bacc.Bacc(
        "TRN2", target_bir_lowering=False, debug=False, num_devices=NCORES
    )
    xt_h = nc.dram_tensor("xt", [C, n], f16, kind="ExternalInput")
    at_h = nc.dram_tensor("at", [nch, P, nt * free], u8,
                          kind="ExternalInput")  # [8, 128, 32*512]
    w_h = nc.dram_tensor("wt", [7, P, P], f16, kind="ExternalInput")
    out_h = nc.dram_tensor("out", [C, n], f16, kind="ExternalOutput")

    with tile.TileContext(nc) as tc:
        with (
            tc.tile_pool(name="const", bufs=1) as constp,
            tc.tile_pool(name="htp", bufs=2) as htp,
            tc.tile_pool(name="g8p", bufs=2) as g8p,
            tc.tile_pool(name="outp", bufs=4) as outp,
            tc.tile_pool(name="lkp", bufs=4) as lkp,
            tc.tile_pool(name="psp", bufs=3, space="PSUM") as psp,
        ):
            # One DMA queue (Sync), priority order: X^T halves, weights,
            # then the 8 column panels. (Multi-queue splits the per-core
            # HBM port bandwidth, it does not add any.)
            # Weight slots: 0: S*W0 (prepass -> PSUM = S*(X W0), H kept
            # as S*H); 1-3: W1/S, W2/S, I/S (tiny path (S*H)(W/S) = G);
            # 4-6: W1, W2, I (identity path (S*H) W = S*G).
            xt_sb = constp.tile([C, n], f16)
            h = n // 2
            nc.sync.dma_start(xt_sb[:, :h], xt_h[:, :h])
            nc.sync.dma_start(xt_sb[:, h:], xt_h[:, h:])
            w_sb = constp.tile([P, 7, P], f16)
            nc.sync.dma_start(w_sb[:], w_h[:].rearrange("w p q -> p w q"))

            at_res = [
                constp.tile([P, nt, free], u8, name=f"atc{j}")
                for j in range(nch)
            ]
            for j in range(nch):
                nc.sync.dma_start(
                    at_res[j][:],
                    at_h[j].rearrange("p (t c) -> p t c", t=nt),
                )

            def pan(c, p):  # fp8 view: panel c, DoubleRow k-pair p
                return at_res[c].bitcast(f8)[:, 2 * p:2 * p + 2, :]

            def lrelu(dest, ps):
                # dest = max(NEG*t, t), t = fp16 copy of ps (= S*Z).
                # (single-op stt from PSUM is illegal: only one PSUM
                # input per instruction; SBUF fp16 gets 2x DVE rate.)
                t = lkp.tile([P, dest.shape[-1]], f16, tag="lk", name="lk")
                nc.scalar.copy(t[:], ps[:])
                nc.vector.scalar_tensor_tensor(
                    dest, t[:], NEG_SLOPE, t[:], ALU.mult, ALU.max
                )

            def make_g8(ht, w_idx, ncx, g8_dst):
                # G tiles (N-layout fp8) for chunk ncx: tpc transpose-
                # matmuls into one psum bank + one wide cast-copy.
                pst = psp.tile([P, tpc, P], f32, tag="ps", name="pst")
                for j in range(tpc):
                    mt = ncx * tpc + j
                    nc.tensor.matmul(
                        pst[:, j, :], ht[:, mt * P:(mt + 1) * P],
                        w_sb[:, w_idx, :], start=True, stop=True,
                    )
                dst = g8_dst[:, ncx * tpc:(ncx + 1) * tpc, :]
                if ncx % 2 == 0:
                    nc.vector.tensor_copy(dst, pst[:])
                else:
                    nc.scalar.copy(dst, pst[:])

            # ---- prepass: H0^T = S*lrelu(X W0) (T-layout, fp16) ----
            ht0 = htp.tile([C, n], f16, tag="ht", name="ht0")
            for ch in range(nch):
                sl = slice(ch * free, (ch + 1) * free)
                ps = psp.tile([P, free], f32, tag="ps", name="psx")
                nc.tensor.matmul(
                    ps[:], w_sb[:, 0, :], xt_sb[:, sl], start=True, stop=True
                )
                lrelu(ht0[:, sl], ps)

            g8_0 = g8p.tile([P, nt, P], f8, tag="g8", name="g80")
            for ncx in range(nch):
                make_g8(ht0, 1, ncx, g8_0)

            # ---- layer 0 stream + layer 1 wavefront ----
            ht1 = htp.tile([C, n], f16, tag="ht", name="ht1")
            g8_1 = g8p.tile([P, nt, P], f8, tag="g8", name="g81")
            ht2 = htp.tile([C, n], f16, tag="ht", name="ht2")
            g8_2 = g8p.tile([P, nt, P], f8, tag="g8", name="g82")

            l1_ps = [None] * nch   # L1 accumulators (banks < nwb persistent)
            issued = [0] * nch     # pairs accumulated per L1 bank
            ps_l0 = [None] * nch

            def issue_chunk(ps, g8_src, w_id, ht_src, c):
                # full accumulation for one chunk: identity + all pairs
                sl = slice(c * free, (c + 1) * free)
                nc.tensor.matmul(
                    ps[:], w_sb[:, w_id, :], ht_src[:, sl],
                    start=True, stop=False,
                )
                for p in range(npr):
                    nc.tensor.matmul(
                        ps[:], g8_src[:, 2 * p:2 * p + 2, :], pan(c, p),
                        perf_mode=DR, start=False, stop=(p == npr - 1),
                    )

            def finish_l0(j):
                sl = slice(j * free, (j + 1) * free)
                lrelu(ht1[:, sl], ps_l0[j])
                make_g8(ht1, 2, j, g8_1)   # -> g8_1 pairs 2j, 2j+1

            for j in range(nch):
                ps_l0[j] = psp.tile([P, free], f32, tag="ps", name=f"ps0c{j}")
                issue_chunk(ps_l0[j], g8_0, 4, ht0, j)
                if j >= 1:
                    finish_l0(j - 1)
                    # L1 wavefront: pairs 0..2j-1 exist; panels 0..j-1
                    # arrived. Open bank c with its identity matmul,
                    # then catch up to all available pairs.
                    avail = 2 * j
                    for c in range(min(j, nwb)):
                        if issued[c] == 0:
                            l1_ps[c] = psp.tile(
                                [P, free], f32, tag="acc", name=f"ps1c{c}"
                            )
                            sl = slice(c * free, (c + 1) * free)
                            nc.tensor.matmul(
                                l1_ps[c][:], w_sb[:, 5, :], ht1[:, sl],
                                start=True, stop=False,
                            )
                        while issued[c] < avail:
                            p = issued[c]
                            issued[c] += 1
                            nc.tensor.matmul(
                                l1_ps[c][:], g8_1[:, 2 * p:2 * p + 2, :],
                                pan(c, p), perf_mode=DR,
                                start=False, stop=False,
                            )
            finish_l0(nch - 1)

            # ---- layer 1 epilogue ----
            def finish_l1(c):
                sl = slice(c * free, (c + 1) * free)
                lrelu(ht2[:, sl], l1_ps[c])
                make_g8(ht2, 3, c, g8_2)

            for c in range(nwb):   # drain wavefront banks
                while issued[c] < npr:
                    p = issued[c]
                    issued[c] += 1
                    nc.tensor.matmul(
                        l1_ps[c][:], g8_1[:, 2 * p:2 * p + 2, :], pan(c, p),
                        perf_mode=DR, start=False, stop=(p == npr - 1),
                    )
                if c >= 1:
                    finish_l1(c - 1)
            for c in range(nwb, nch):   # remaining chunks, chunk-major
                l1_ps[c] = psp.tile([P, free], f32, tag="acc", name=f"ps1t{c}")
                issue_chunk(l1_ps[c], g8_1, 5, ht1, c)
                finish_l1(c - 1)
            finish_l1(nch - 1)

            # ---- layer 2, chunk-major, output streamed out ----
            l2_ps = [None] * nch

            def finish_l2(c):
                sl = slice(c * free, (c + 1) * free)
                oc = outp.tile([P, free], f16, tag="oc", name="oc")
                if c % 2 == 0:
                    nc.vector.tensor_scalar_mul(oc[:], l2_ps[c][:], 1.0 / SCALE)
                else:
                    nc.scalar.mul(oc[:], l2_ps[c][:], 1.0 / SCALE)
                nc.sync.dma_start(out_h[:, sl], oc[:])

            for c in range(nch):
                l2_ps[c] = psp.tile([P, free], f32, tag="acc", name=f"ps2c{c}")
                issue_chunk(l2_ps[c], g8_2, 6, ht2, c)
                if c >= 1:
                    finish_l2(c - 1)
            finish_l2(nch - 1)

    nc.compile()
    return nc


def _get_nc(n, free, use_double_row=True, wavefront=5):
    key = (n, free, use_double_row, wavefront)
    if key not in _CACHE:
        _CACHE[key] = _build_nc(n, free, use_double_row, wavefront)
    return _CACHE[key]


def _block_diag(w, reps):
    d = w.shape[0]
    out = np.zeros((reps * d, reps * d), dtype=np.float32)
    for b in range(reps):
        out[b * d:(b + 1) * d, b * d:(b + 1) * d] = w
    return out


def prepare_inputs(x, adj, Identity, W0, W1, W2, n=N_FULL, free=512):
    """Host-side layout prep. Returns per-core input maps."""
    import ml_dtypes

    b_full = x.shape[0]
    b_core = b_full // NCORES
    c = b_core * D
    nch = n // free
    nt = n // P

    a8 = (
        np.ascontiguousarray(adj.T.astype(np.float32)) * SCALE
    ).astype(ml_dtypes.float8_e4m3).view(np.uint8)      # [k, m]
    # column panels, pre-tiled [panel, partition, k-tile, col] so each
    # panel is one fully contiguous per-partition DMA
    at8 = np.empty((nch, P, nt * free), dtype=np.uint8)
    for j in range(nch):
        blk = a8[:, j * free:(j + 1) * free]            # [n, free]
        at8[j] = (
            blk.reshape(nt, P, free).transpose(1, 0, 2).reshape(P, nt * free)
        )

    reps = c // D
    wb = [
        _block_diag(np.asarray(W, np.float32), reps) for W in (W0, W1, W2)
    ]
    eye = np.eye(c, dtype=np.float32)
    w_all = np.stack(
        [SCALE * wb[0], wb[1] / SCALE, wb[2] / SCALE, eye / SCALE,
         wb[1], wb[2], eye]
    ).astype(np.float16)

    xf = np.asarray(x, np.float32)
    in_maps = []
    for core in range(NCORES):
        xs = xf[core * b_core:(core + 1) * b_core]      # (b_core, n, D)
        xt = np.ascontiguousarray(
            xs.transpose(0, 2, 1).reshape(c, n)
        ).astype(np.float16)
        in_maps.append({"xt": xt, "at": at8, "wt": w_all})
    return in_maps


def gather_output(results, n=N_FULL, b_full=B_FULL):
    b_core = b_full // NCORES
    out = np.empty((b_full, n, D), dtype=np.float32)
    for core in range(NCORES):
        oc = np.asarray(results[core]["out"], np.float32).reshape(b_core, D, n)
        out[core * b_core:(core + 1) * b_core] = oc.transpose(0, 2, 1)
    return out


def run(x, adj, Identity, W0, W1, W2, n=N_FULL, free=512, trace=False,
        use_double_row=True, wavefront=5, **_ignored):
    from concourse.bass_utils import run_bass_kernel_spmd

    nc = _get_nc(n, free, use_double_row, wavefront)
    in_maps = prepare_inputs(x, adj, Identity, W0, W1, W2, n, free)
    core_ids = list(range(NCORES))
    res = run_bass_kernel_spmd(nc, in_maps, core_ids, trace=trace)
    out = gather_output(res.results, n, x.shape[0])
    return out, res


def kernel(x, adj, Identity, W0, W1, W2):
    out, _ = run(x, adj, Identity, W0, W1, W2)
    return out
